# revision 16
# baseline (speedup 1.0000x reference)
"""Trainium2 Bass kernel for nn_ContrastiveLoss3DTo2D.

Reference computation (B=256, D=1024, margin=0.2):
    scores[i, j] = dot(im[j], s[i, j])                    # [B, B]
    cost_s  = sum_i relu(margin + max_{j!=i} scores[i,j] - scores[i,i])
    cost_im = sum_j relu(margin + max_{i!=j} scores[i,j] - scores[j,j])
    loss = cost_s + cost_im

Sharding: s (and the score matrix) is sharded along i across 8 cores
(32 rows each); im is replicated. Each core streams its 32 MB shard from
HBM at the DMA-engine roofline (~410 GB/s) on the sync HWDGE ring while
im + masks ride the scalar ring. Scores are produced by ONE fused
multiply+accumulate pass per row-half:
  DVE  scalar_tensor_tensor  (out -> stride-0 dummy, accum -> scoresT col)
  Pool tensor_mul + ACT accumulate for ~1 row per steady chunk
so no second full-tensor pass exists on any single engine and compute
hides fully under the stream. Chunks ramp up [1,1,2] and down [2,1,1] so
the first fused op starts early and the last row's compute tail is
minimal. The epilogue does masked column/row maxima + diagonal extraction
on tiny [128,32]/[32,256] tiles after a 32x32 block transpose.
Per-core outputs are tiny partials:
  rowout[32,2]  - (diag, rowcost) per local row
  colout[128,2] - per-column max over local rows (diag masked), col t*128+p
The host combines partials (max over cores for columns, sums) into the
scalar loss. relu/max commute (relu is monotone), so per-core column
maxima compose exactly.
"""

import numpy as np

B = 256
D = 1024
M = 8            # cores
BL = B // M      # 32 local rows per core
P = 128          # SBUF partitions
T = B // P       # 2 column tiles of 128
MARGIN = 0.2
NEG = -1.0e30    # diagonal mask value

# Per-row compute path. Measured per-row costs under the full DMA stream:
#   stt : 3.7 us DVE (fused multiply+accumulate, no other engine)
#   mul : 2.2 us DVE (plain multiply) + 2.8 us ACT (accumulate)
#   pool: 5.1 us Pool (multiply) + 2.8 us ACT (accumulate)
# The mix keeps every engine at ~60-70 us, under the ~82 us stream, and
# the tail rows use the lowest-latency paths available at that moment.
_RAMP = {0: "stt", 1: "mul", 2: "pool", 3: "mul"}
_TAIL = {28: "pool", 29: "mul", 30: "stt", 31: "stt"}


def _row_path(i):
    if i in _RAMP:
        return _RAMP[i]
    if i in _TAIL:
        return _TAIL[i]
    chunk, r = divmod(i - 4, 3)
    # First two steady chunks lean on ACT (mul,mul,pool); the rest are
    # (stt,mul,pool) so the late stretch is light on ACT.
    if chunk < 2:
        return ("mul", "mul", "pool")[r]
    return ("stt", "mul", "pool")[r]

_NC = None


def _build_nc():
    import concourse.bacc as bacc
    from concourse import mybir
    from concourse.tile import TileContext

    f32 = mybir.dt.float32
    add = mybir.AluOpType.add
    mult = mybir.AluOpType.mult
    amax = mybir.AluOpType.max

    nc = bacc.Bacc(None, target_bir_lowering=False, debug=False)
    im_d = nc.declare_dram_parameter("im", [B, D], f32, isOutput=False)
    s_d = nc.declare_dram_parameter("s", [BL, B, D], f32, isOutput=False)
    mt_d = nc.declare_dram_parameter("mask_t_neg", [B, BL], f32, isOutput=False)
    rm_d = nc.declare_dram_parameter("row_masks", [BL, 2 * B], f32, isOutput=False)
    ro_d = nc.declare_dram_parameter("rowout", [BL, 2], f32, isOutput=True)
    co_d = nc.declare_dram_parameter("colout", [P, T], f32, isOutput=True)

    with TileContext(nc) as tc:
        with (
            tc.tile_pool(name="const", bufs=1) as cpool,
            tc.tile_pool(name="sload", bufs=7) as spool,
            tc.tile_pool(name="scratch", bufs=2) as prpool,
            tc.tile_pool(name="gprod", bufs=2) as gpool,
            tc.tile_pool(name="vprod", bufs=2, space="PSUM") as vpool,
            tc.tile_pool(name="small", bufs=1) as smpool,
            tc.tile_pool(name="psc", bufs=1, space="PSUM") as ppool,
        ):
            # im packed as [p, t*D + d] so j = t*128 + p matches the s tiles.
            # Rides the scalar (ACT) HWDGE ring so the s stream owns sync.
            im_t = cpool.tile([P, T * D], f32, tag="im")
            nc.scalar.dma_start(
                out=im_t[:].rearrange("p (t d) -> p t d", t=T),
                in_=im_d[:].rearrange("(t p) d -> p t d", p=P),
            )
            # Epilogue masks also on the scalar ring.
            mt_t = cpool.tile([P, T * BL], f32, tag="maskT")
            nc.scalar.dma_start(
                out=mt_t[:].rearrange("p (t i) -> p t i", t=T),
                in_=mt_d[:].rearrange("(t p) i -> p t i", p=P),
            )
            rm_t = cpool.tile([BL, 2 * B], f32, tag="rowmasks")
            nc.scalar.dma_start(out=rm_t[:], in_=rm_d[:])
            er_t = rm_t[:, 0:B]       # 1.0 at own-diagonal column
            nr_t = rm_t[:, B:2 * B]   # NEG at own-diagonal column

            # scores^T: partition = column j (within tile t), free = t*BL + i
            scoresT = smpool.tile([P, T * BL], f32, tag="scoresT")

            # Stride-0 garbage sinks for the fused ops' full-width out.
            dumP = smpool.tile([P, 1], f32, tag="dumP")
            dumB = smpool.tile([BL, 1], f32, tag="dumB")
            # ACT's accumulate pass writes its product copy into PSUM.
            pscr_a = ppool.tile([P, D], f32, tag="pscr_a")

            # Ramped chunk sizes: small first chunks so the first fused op
            # starts as soon as ~1 MB has landed; single-row tail chunks so
            # the final rows' compute latency is minimal.
            chunk_rows = [1, 1, 2] + [3] * 8 + [1, 1, 1, 1]
            assert sum(chunk_rows) == BL

            def emit_act_accum(prod, i):
                # Accumulate a [P, 2D] product into scoresT on the ACT engine.
                for t in range(T):
                    nc.scalar.activation(
                        out=pscr_a[:],
                        in_=prod[:, t * D:(t + 1) * D],
                        func=mybir.ActivationFunctionType.Copy,
                        accum_out=scoresT[:, t * BL + i:t * BL + i + 1],
                    )

            deferred_act = []  # (prod_tile, row) pairs flushed later
            row0 = 0
            for nr in chunk_rows:
                s_t = spool.tile([P, nr * T * D], f32, tag="s")
                nc.sync.dma_start(
                    out=s_t[:, 0:nr * T * D].rearrange(
                        "p (r t d) -> p r t d", r=nr, t=T
                    ),
                    in_=s_d[row0:row0 + nr].rearrange(
                        "r (t p) d -> p r t d", p=P
                    ),
                )
                for r in range(nr):
                    i = row0 + r
                    off = r * T * D
                    path = _row_path(i)
                    if path == "pool":
                        prod_g = gpool.tile([P, T * D], f32, tag="prod_g")
                        nc.gpsimd.tensor_mul(
                            prod_g[:], s_t[:, off:off + T * D], im_t[:]
                        )
                        if i in _TAIL:
                            # The pool multiply finishes after the next mul
                            # row's product: emit its ACT accums after that
                            # row's so ACT isn't head-of-line blocked.
                            deferred_act.append((prod_g, i))
                        else:
                            emit_act_accum(prod_g, i)
                    elif path == "mul":
                        # Product halves go to PSUM: no SBUF write traffic
                        # from DVE, no SBUF read traffic from ACT.
                        for t in range(T):
                            prod_v = vpool.tile([P, D], f32, tag="prod_v")
                            nc.vector.tensor_mul(
                                prod_v[:],
                                s_t[:, off + t * D:off + (t + 1) * D],
                                im_t[:, t * D:(t + 1) * D],
                            )
                            nc.scalar.activation(
                                out=pscr_a[:],
                                in_=prod_v[:],
                                func=mybir.ActivationFunctionType.Copy,
                                accum_out=scoresT[:, t * BL + i:t * BL + i + 1],
                            )
                        for pg, ig in deferred_act:
                            emit_act_accum(pg, ig)
                        deferred_act.clear()
                    else:
                        for t in range(T):
                            nc.vector.scalar_tensor_tensor(
                                out=dumP[:].broadcast_to([P, D]),
                                in0=s_t[:, off + t * D:off + (t + 1) * D],
                                scalar=1.0,
                                in1=im_t[:, t * D:(t + 1) * D],
                                op0=mult, op1=mult,
                                accum_out=scoresT[:, t * BL + i:t * BL + i + 1],
                            )
                row0 += nr
            for pg, ig in deferred_act:
                emit_act_accum(pg, ig)
            deferred_act.clear()

            # Column maxima over local rows, diagonal masked to -1e30.
            colmax = smpool.tile([P, T], f32, tag="colmax")
            for t in range(T):
                cscr = prpool.tile([P, BL], f32, tag="cscr")
                nc.vector.tensor_add(
                    cscr[:],
                    scoresT[:, t * BL:(t + 1) * BL],
                    mt_t[:, t * BL:(t + 1) * BL],
                )
                nc.vector.reduce_max(
                    colmax[:, t:t + 1], cscr[:], axis=mybir.AxisListType.X
                )

            # Transpose scores^T -> rows [32, 256] via 32x32 stream blocks.
            rows = smpool.tile([BL, B], f32, tag="rows")
            for t in range(T):
                for k in range(P // 32):
                    nc.vector.transpose(
                        out=rows[0:BL, t * P + k * 32:t * P + (k + 1) * 32],
                        in_=scoresT[k * 32:(k + 1) * 32, t * BL:(t + 1) * BL],
                    )

            # rowout col 0 = diag, col 1 = relu(margin + rowmax_offdiag - diag)
            rowstat = smpool.tile([BL, 4], f32, tag="rowstat")
            nc.vector.scalar_tensor_tensor(
                out=dumB[:].broadcast_to([BL, B]),
                in0=rows[:], scalar=1.0, in1=er_t,
                op0=mult, op1=mult,
                accum_out=rowstat[:, 0:1],
            )
            rscr = prpool.tile([BL, B], f32, tag="rscr")
            nc.vector.tensor_add(rscr[:], rows[:], nr_t)
            nc.vector.reduce_max(
                rowstat[:, 2:3], rscr[:], axis=mybir.AxisListType.X
            )
            nc.vector.tensor_sub(rowstat[:, 3:4], rowstat[:, 2:3], rowstat[:, 0:1])
            nc.vector.tensor_scalar(
                out=rowstat[:, 1:2], in0=rowstat[:, 3:4],
                scalar1=MARGIN, scalar2=0.0, op0=add, op1=amax,
            )

            nc.sync.dma_start(out=ro_d[:], in_=rowstat[:, 0:2])
            nc.sync.dma_start(out=co_d[:], in_=colmax[:])

    nc.compile()
    return nc


def _get_nc():
    global _NC
    if _NC is None:
        _NC = _build_nc()
    return _NC


def _make_in_maps(im, s):
    il = np.arange(BL)
    in_maps = []
    for c in range(M):
        mt = np.zeros((B, BL), np.float32)
        rm = np.zeros((BL, 2 * B), np.float32)
        mt[c * BL + il, il] = NEG
        rm[il, c * BL + il] = 1.0        # er: extract own diagonal
        rm[il, B + c * BL + il] = NEG    # nr: mask own diagonal for rowmax
        in_maps.append({
            "im": im,
            "s": s[c * BL:(c + 1) * BL],
            "mask_t_neg": mt,
            "row_masks": rm,
        })
    return in_maps


def _combine(results):
    diag = np.concatenate([results[c]["rowout"][:, 0] for c in range(M)])
    rowcosts = np.concatenate([results[c]["rowout"][:, 1] for c in range(M)])
    # colout[p, t] is the per-core max for column j = t*128 + p.
    colmax = np.max(
        np.stack([results[c]["colout"].T.reshape(-1) for c in range(M)]),
        axis=0,
    )
    cost_im = np.maximum(np.float32(MARGIN) + colmax - diag, np.float32(0.0))
    loss = rowcosts.sum(dtype=np.float32) + cost_im.sum(dtype=np.float32)
    return np.array(loss, dtype=np.float32)


def _run(im, s, **spmd_kwargs):
    from concourse.bass_utils import run_bass_kernel_spmd

    im = np.ascontiguousarray(np.asarray(im), dtype=np.float32)
    s = np.ascontiguousarray(np.asarray(s), dtype=np.float32)
    nc = _get_nc()
    res = run_bass_kernel_spmd(nc, _make_in_maps(im, s), list(range(M)),
                               **spmd_kwargs)
    return _combine(res.results), res


def kernel(im, s):
    loss, _ = _run(im, s)
    return loss


# revision 20
# speedup vs baseline: 1.0558x; 1.0558x over previous
"""Trainium2 Bass kernel for nn_ContrastiveLoss3DTo2D.

Reference computation (B=256, D=1024, margin=0.2):
    scores[i, j] = dot(im[j], s[i, j])                    # [B, B]
    cost_s  = sum_i relu(margin + max_{j!=i} scores[i,j] - scores[i,i])
    cost_im = sum_j relu(margin + max_{i!=j} scores[i,j] - scores[j,j])
    loss = cost_s + cost_im

Sharding: s (and the score matrix) is sharded along i across 8 cores
(32 rows each); im is replicated. Each core streams its 32 MB shard from
HBM at the DMA-engine roofline (~410 GB/s) on the sync HWDGE ring while
im + masks ride the scalar ring. Scores are produced by ONE fused
multiply+accumulate pass per row-half:
  DVE  scalar_tensor_tensor  (out -> stride-0 dummy, accum -> scoresT col)
  Pool tensor_mul + ACT accumulate for ~1 row per steady chunk
so no second full-tensor pass exists on any single engine and compute
hides fully under the stream. Chunks ramp up [1,1,2] and down [2,1,1] so
the first fused op starts early and the last row's compute tail is
minimal. The epilogue does masked column/row maxima + diagonal extraction
on tiny [128,32]/[32,256] tiles after a 32x32 block transpose.
Per-core outputs are tiny partials:
  rowout[32,2]  - (diag, rowcost) per local row
  colout[128,2] - per-column max over local rows (diag masked), col t*128+p
The host combines partials (max over cores for columns, sums) into the
scalar loss. relu/max commute (relu is monotone), so per-core column
maxima compose exactly.
"""

import numpy as np

B = 256
D = 1024
M = 8            # cores
BL = B // M      # 32 local rows per core
P = 128          # SBUF partitions
T = B // P       # 2 column tiles of 128
MARGIN = 0.2
NEG = -1.0e30    # diagonal mask value

# Per-row compute path. Measured per-row costs under the full DMA stream:
#   stt : 3.7 us DVE (fused multiply+accumulate, no other engine)
#   mul : 2.2 us DVE (plain multiply) + 2.8 us ACT (accumulate)
#   pool: 5.1 us Pool (multiply) + 2.8 us ACT (accumulate)
# The mix keeps every engine at ~60-70 us, under the ~82 us stream, and
# the tail rows use the lowest-latency paths available at that moment.
_RAMP = {0: "stt", 1: "mul", 2: "pool", 3: "mul"}
_TAIL = {28: "mul", 29: "stt", 30: "stt", 31: "stt"}


def _row_path(i):
    if i in _RAMP:
        return _RAMP[i]
    if i in _TAIL:
        return _TAIL[i]
    return ("stt", "mul", "pool")[(i - 4) % 3]

_NC = None


def _build_nc():
    import concourse.bacc as bacc
    from concourse import mybir
    from concourse.tile import TileContext

    f32 = mybir.dt.float32
    add = mybir.AluOpType.add
    mult = mybir.AluOpType.mult
    amax = mybir.AluOpType.max

    nc = bacc.Bacc(None, target_bir_lowering=False, debug=False)
    im_d = nc.declare_dram_parameter("im", [B, D], f32, isOutput=False)
    s_d = nc.declare_dram_parameter("s", [BL, B, D], f32, isOutput=False)
    mt_d = nc.declare_dram_parameter("mask_t_neg", [B, BL], f32, isOutput=False)
    rm_d = nc.declare_dram_parameter("row_masks", [BL, 2 * B], f32, isOutput=False)
    ro_d = nc.declare_dram_parameter("rowout", [BL, 2], f32, isOutput=True)
    co_d = nc.declare_dram_parameter("colout", [P, T], f32, isOutput=True)

    with TileContext(nc) as tc:
        with (
            tc.tile_pool(name="const", bufs=1) as cpool,
            tc.tile_pool(name="sload", bufs=7) as spool,
            tc.tile_pool(name="scratch", bufs=2) as prpool,
            tc.tile_pool(name="gprod", bufs=3) as gpool,
            tc.tile_pool(name="vprod", bufs=2, space="PSUM") as vpool,
            tc.tile_pool(name="small", bufs=1) as smpool,
            tc.tile_pool(name="psc", bufs=1, space="PSUM") as ppool,
        ):
            # im packed as [p, t*D + d] so j = t*128 + p matches the s tiles.
            # First on the sync ring, ahead of the s stream: the DMA engines
            # are shared so total stream time is unchanged, but compute can
            # start as soon as chunk 0 lands.
            im_t = cpool.tile([P, T * D], f32, tag="im")
            nc.sync.dma_start(
                out=im_t[:].rearrange("p (t d) -> p t d", t=T),
                in_=im_d[:].rearrange("(t p) d -> p t d", p=P),
            )
            # Epilogue masks also on the scalar ring.
            mt_t = cpool.tile([P, T * BL], f32, tag="maskT")
            nc.scalar.dma_start(
                out=mt_t[:].rearrange("p (t i) -> p t i", t=T),
                in_=mt_d[:].rearrange("(t p) i -> p t i", p=P),
            )
            rm_t = cpool.tile([BL, 2 * B], f32, tag="rowmasks")
            nc.scalar.dma_start(out=rm_t[:], in_=rm_d[:])
            er_t = rm_t[:, 0:B]       # 1.0 at own-diagonal column
            nr_t = rm_t[:, B:2 * B]   # NEG at own-diagonal column

            # scores^T: partition = column j (within tile t), free = t*BL + i
            scoresT = smpool.tile([P, T * BL], f32, tag="scoresT")

            # Stride-0 garbage sinks for the fused ops' full-width out.
            dumP = smpool.tile([P, 1], f32, tag="dumP")
            dumB = smpool.tile([BL, 1], f32, tag="dumB")
            # ACT's accumulate pass writes its product copy into PSUM.
            pscr_a = ppool.tile([P, D], f32, tag="pscr_a")

            # Ramped chunk sizes: small first chunks so the first fused op
            # starts as soon as ~1 MB has landed; single-row tail chunks so
            # the final rows' compute latency is minimal.
            chunk_rows = [1, 1, 2] + [3] * 8 + [1, 1, 1, 1]
            assert sum(chunk_rows) == BL

            def emit_act_accum(prod, i):
                # Accumulate a [P, 2D] product into scoresT on the ACT engine.
                for t in range(T):
                    nc.scalar.activation(
                        out=pscr_a[:],
                        in_=prod[:, t * D:(t + 1) * D],
                        func=mybir.ActivationFunctionType.Copy,
                        accum_out=scoresT[:, t * BL + i:t * BL + i + 1],
                    )

            deferred_act = []  # (prod_tile, row) pairs flushed later
            row0 = 0
            for nr in chunk_rows:
                s_t = spool.tile([P, nr * T * D], f32, tag="s")
                nc.sync.dma_start(
                    out=s_t[:, 0:nr * T * D].rearrange(
                        "p (r t d) -> p r t d", r=nr, t=T
                    ),
                    in_=s_d[row0:row0 + nr].rearrange(
                        "r (t p) d -> p r t d", p=P
                    ),
                )
                for r in range(nr):
                    i = row0 + r
                    off = r * T * D
                    path = _row_path(i)
                    if path == "pool":
                        prod_g = gpool.tile([P, T * D], f32, tag="prod_g")
                        nc.gpsimd.tensor_mul(
                            prod_g[:], s_t[:, off:off + T * D], im_t[:]
                        )
                        # The pool multiply is slow; emit its ACT accums
                        # after the NEXT mul row's (one chunk later) so ACT
                        # is never head-of-line blocked on the pool product.
                        deferred_act.append((prod_g, i))
                    elif path == "mul":
                        # Product halves go to PSUM: no SBUF write traffic
                        # from DVE, no SBUF read traffic from ACT.
                        for t in range(T):
                            prod_v = vpool.tile([P, D], f32, tag="prod_v")
                            nc.vector.tensor_mul(
                                prod_v[:],
                                s_t[:, off + t * D:off + (t + 1) * D],
                                im_t[:, t * D:(t + 1) * D],
                            )
                            nc.scalar.activation(
                                out=pscr_a[:],
                                in_=prod_v[:],
                                func=mybir.ActivationFunctionType.Copy,
                                accum_out=scoresT[:, t * BL + i:t * BL + i + 1],
                            )
                        for pg, ig in deferred_act:
                            emit_act_accum(pg, ig)
                        deferred_act.clear()
                    else:
                        for t in range(T):
                            nc.vector.scalar_tensor_tensor(
                                out=dumP[:].broadcast_to([P, D]),
                                in0=s_t[:, off + t * D:off + (t + 1) * D],
                                scalar=1.0,
                                in1=im_t[:, t * D:(t + 1) * D],
                                op0=mult, op1=mult,
                                accum_out=scoresT[:, t * BL + i:t * BL + i + 1],
                            )
                row0 += nr
            for pg, ig in deferred_act:
                emit_act_accum(pg, ig)
            deferred_act.clear()

            # Column maxima over local rows, diagonal masked to -1e30.
            colmax = smpool.tile([P, T], f32, tag="colmax")
            for t in range(T):
                cscr = prpool.tile([P, BL], f32, tag="cscr")
                nc.vector.tensor_add(
                    cscr[:],
                    scoresT[:, t * BL:(t + 1) * BL],
                    mt_t[:, t * BL:(t + 1) * BL],
                )
                nc.vector.reduce_max(
                    colmax[:, t:t + 1], cscr[:], axis=mybir.AxisListType.X
                )

            # Transpose scores^T -> rows [32, 256] via 32x32 stream blocks.
            rows = smpool.tile([BL, B], f32, tag="rows")
            for t in range(T):
                for k in range(P // 32):
                    nc.vector.transpose(
                        out=rows[0:BL, t * P + k * 32:t * P + (k + 1) * 32],
                        in_=scoresT[k * 32:(k + 1) * 32, t * BL:(t + 1) * BL],
                    )

            # rowout col 0 = diag, col 1 = relu(margin + rowmax_offdiag - diag)
            rowstat = smpool.tile([BL, 4], f32, tag="rowstat")
            nc.vector.scalar_tensor_tensor(
                out=dumB[:].broadcast_to([BL, B]),
                in0=rows[:], scalar=1.0, in1=er_t,
                op0=mult, op1=mult,
                accum_out=rowstat[:, 0:1],
            )
            rscr = prpool.tile([BL, B], f32, tag="rscr")
            nc.vector.tensor_add(rscr[:], rows[:], nr_t)
            nc.vector.reduce_max(
                rowstat[:, 2:3], rscr[:], axis=mybir.AxisListType.X
            )
            nc.vector.tensor_sub(rowstat[:, 3:4], rowstat[:, 2:3], rowstat[:, 0:1])
            nc.vector.tensor_scalar(
                out=rowstat[:, 1:2], in0=rowstat[:, 3:4],
                scalar1=MARGIN, scalar2=0.0, op0=add, op1=amax,
            )

            nc.sync.dma_start(out=ro_d[:], in_=rowstat[:, 0:2])
            nc.sync.dma_start(out=co_d[:], in_=colmax[:])

    nc.compile()
    return nc


def _get_nc():
    global _NC
    if _NC is None:
        _NC = _build_nc()
    return _NC


def _make_in_maps(im, s):
    il = np.arange(BL)
    in_maps = []
    for c in range(M):
        mt = np.zeros((B, BL), np.float32)
        rm = np.zeros((BL, 2 * B), np.float32)
        mt[c * BL + il, il] = NEG
        rm[il, c * BL + il] = 1.0        # er: extract own diagonal
        rm[il, B + c * BL + il] = NEG    # nr: mask own diagonal for rowmax
        in_maps.append({
            "im": im,
            "s": s[c * BL:(c + 1) * BL],
            "mask_t_neg": mt,
            "row_masks": rm,
        })
    return in_maps


def _combine(results):
    diag = np.concatenate([results[c]["rowout"][:, 0] for c in range(M)])
    rowcosts = np.concatenate([results[c]["rowout"][:, 1] for c in range(M)])
    # colout[p, t] is the per-core max for column j = t*128 + p.
    colmax = np.max(
        np.stack([results[c]["colout"].T.reshape(-1) for c in range(M)]),
        axis=0,
    )
    cost_im = np.maximum(np.float32(MARGIN) + colmax - diag, np.float32(0.0))
    loss = rowcosts.sum(dtype=np.float32) + cost_im.sum(dtype=np.float32)
    return np.array(loss, dtype=np.float32)


def _run(im, s, **spmd_kwargs):
    from concourse.bass_utils import run_bass_kernel_spmd

    im = np.ascontiguousarray(np.asarray(im), dtype=np.float32)
    s = np.ascontiguousarray(np.asarray(s), dtype=np.float32)
    nc = _get_nc()
    res = run_bass_kernel_spmd(nc, _make_in_maps(im, s), list(range(M)),
                               **spmd_kwargs)
    return _combine(res.results), res


def kernel(im, s):
    loss, _ = _run(im, s)
    return loss
